# revision 1
# baseline (speedup 1.0000x reference)
"""Row-parallel masked-diagonal attention for Trainium2 (8 NeuronCores).

Problem: nn_DotProductAttention — N=8192, D=128 fp32.
    score   = (Q @ V^T) * (1 - eye(N))
    attn    = softmax(score, axis=-1)
    context = attn @ V
    returns (context, attn)

Strategy (per core c, rows [1024c, 1024c+1024)):
  - Host rotates V by -1024c rows for core c, so the diagonal of the score
    block always falls at static column offset 128*t for m-tile t (the host
    un-rotates attn columns afterwards; context is rotation-invariant).
  - S = Q V^T computed in 3 float32r passes (Qr·Vr + dQ·Vr + Qr·dV), which
    recovers ~fp32 accuracy at 1 PE cycle/row instead of fp32's 4.
  - exp on ACT directly from PSUM with accum_out giving row sums for free.
  - normalize in place on DVE (per-partition 1/rowsum).
  - A^T tiles via PE transpose (fp32) -> PSUM -> DVE copy (rounds to f32r)
  - context^T accumulated over 64 n-chunks in PSUM (f32r matmuls, free dim
    256 so they run at 1 cycle/row), then PE-transposed back to [m, d].
"""

import sys

sys.path.insert(0, "/opt/trn_rl_repo")

from contextlib import ExitStack

import numpy as np

import concourse.bass as bass
import concourse.tile as tile
from concourse import bacc, mybir
from concourse.bass_utils import run_bass_kernel_spmd
from concourse.masks import make_identity

N_CORES = 8
N, D = 8192, 128
P = 128
M_SH = N // N_CORES  # 1024 rows per core
N_MT = M_SH // P  # 8 m-tiles per core
N_CH = N // P  # 64 n-chunks
N_JT = N // 1024  # 8 jtiles (1024-wide S psum tiles)

f32 = mybir.dt.float32
f32r = mybir.dt.float32r
EXP = mybir.ActivationFunctionType.Exp
MUL = mybir.AluOpType.mult
SUB = mybir.AluOpType.subtract
AXX = mybir.AxisListType.X


def _attention(tc, q, v, attn, ctx_out, repeat=1):
    nc = tc.nc
    ctx = ExitStack()
    with ctx:
        persist = ctx.enter_context(tc.tile_pool(name="persist", bufs=1))
        big = ctx.enter_context(tc.tile_pool(name="big", bufs=2))
        stage_p = ctx.enter_context(tc.tile_pool(name="stage", bufs=3))
        small = ctx.enter_context(tc.tile_pool(name="small", bufs=2))
        consts = ctx.enter_context(tc.tile_pool(name="consts", bufs=1))
        spsum = ctx.enter_context(tc.tile_pool(name="spsum", bufs=2, space="PSUM"))
        tpsum = ctx.enter_context(tc.tile_pool(name="tpsum", bufs=2, space="PSUM"))
        cpsum = ctx.enter_context(tc.tile_pool(name="cpsum", bufs=2, space="PSUM"))

        # ---------------- constants ----------------
        ident = consts.tile([P, P], f32, tag="ident")
        make_identity(nc, ident[:])
        ident_r = consts.tile([P, P], f32r, tag="ident_r")
        nc.vector.tensor_copy(ident_r[:], ident[:])
        one_m_i = consts.tile([P, P], f32, tag="onemi")
        nc.vector.memset(one_m_i[:], 1.0)
        nc.vector.tensor_tensor(one_m_i[:], one_m_i[:], ident[:], SUB)

        # ---------------- load + split V ----------------
        # v_nat: [p, c, d] where global row n = 128*c + p
        v_nat = big.tile([P, N_CH, D], f32, tag="bigbuf")
        nc.sync.dma_start(v_nat[:], v.rearrange("(c p) d -> p c d", p=P))
        v_nat_r = persist.tile([P, N_CH, D], f32r, tag="v_nat_r")
        nc.vector.tensor_copy(v_nat_r[:], v_nat[:])
        dv_nat = big.tile([P, N_CH, D], f32r, tag="bigbuf")
        nc.vector.tensor_tensor(dv_nat[:], v_nat[:], v_nat_r[:].bitcast(f32), SUB)

        # V^T parts: [d, n] layout, f32r
        vTr = persist.tile([P, N], f32r, tag="vTr")
        dvT = persist.tile([P, N], f32r, tag="dvT")
        for src, dst in ((v_nat_r, vTr), (dv_nat, dvT)):
            for cg in range(0, N_CH, 4):  # 4 transposes per psum tile
                tp = tpsum.tile([P, 512], f32r, tag="tp_r")
                for k in range(4):
                    c = cg + k
                    nc.tensor.transpose(
                        tp[:, k * P : (k + 1) * P], src[:, c, :], ident_r[:]
                    )
                nc.vector.tensor_copy(dst[:, cg * P : (cg + 4) * P], tp[:])

        # ---------------- load + split Q ----------------
        q_nat = small.tile([P, N_MT, D], f32, tag="q_nat")
        nc.sync.dma_start(q_nat[:], q.rearrange("(t p) d -> p t d", p=P))
        q_nat_r = small.tile([P, N_MT, D], f32r, tag="q_nat_r")
        nc.vector.tensor_copy(q_nat_r[:], q_nat[:])
        dq_nat = small.tile([P, N_MT, D], f32r, tag="dq_nat")
        nc.vector.tensor_tensor(dq_nat[:], q_nat[:], q_nat_r[:].bitcast(f32), SUB)

        qTr = persist.tile([P, N_MT, P], f32r, tag="qTr")
        dqT = persist.tile([P, N_MT, P], f32r, tag="dqT")
        for src, dst in ((q_nat_r, qTr), (dq_nat, dqT)):
            for tg in range(0, N_MT, 4):
                tp = tpsum.tile([P, 512], f32r, tag="tp_r")
                for k in range(4):
                    t = tg + k
                    nc.tensor.transpose(
                        tp[:, k * P : (k + 1) * P], src[:, t, :], ident_r[:]
                    )
                nc.vector.tensor_copy(
                    dst[:, tg : tg + 4, :].rearrange("p t m -> p (t m)"), tp[:]
                )

        attn_t = attn.rearrange("(t p) n -> t p n", p=P)
        ctx_t = ctx_out.rearrange("(t p) d -> t p d", p=P)

        for _rep in range(repeat):
            a_tiles = {}
            for g in range(N_MT // 2):  # m-pair groups
                for t in (2 * g, 2 * g + 1):
                    # ---------------- S = Q V^T (3-pass f32r) ----------------
                    e_tile = big.tile([P, N], f32, tag="bigbuf")
                    partials = small.tile([P, N_JT], f32, tag="partials")
                    for jt in range(N_JT):
                        s_ps = spsum.tile([P, 1024], f32, tag="s_ps")
                        for j in range(2):
                            sl = s_ps[:, j * 512 : (j + 1) * 512]
                            cols = slice(jt * 1024 + j * 512, jt * 1024 + (j + 1) * 512)
                            nc.tensor.matmul(
                                sl, qTr[:, t, :], vTr[:, cols], start=True, stop=False
                            )
                            nc.tensor.matmul(
                                sl, dqT[:, t, :], vTr[:, cols], start=False, stop=False
                            )
                            nc.tensor.matmul(
                                sl, qTr[:, t, :], dvT[:, cols], start=False, stop=True
                            )
                        if jt == 0:
                            # zero the diagonal block (rows t*128+p, col == row)
                            blk = s_ps[:, t * P : (t + 1) * P]
                            nc.vector.tensor_tensor(blk, blk, one_m_i[:], MUL)
                        nc.scalar.activation(
                            e_tile[:, jt * 1024 : (jt + 1) * 1024],
                            s_ps[:],
                            EXP,
                            accum_out=partials[:, jt : jt + 1],
                        )
                    # ---------------- softmax normalize ----------------
                    rowsum = small.tile([P, 1], f32, tag="rowsum")
                    nc.vector.reduce_sum(rowsum[:], partials[:], axis=AXX)
                    rinv = small.tile([P, 1], f32, tag="rinv")
                    nc.vector.reciprocal(rinv[:], rowsum[:])
                    nc.vector.tensor_scalar_mul(e_tile[:], e_tile[:], rinv[:])
                    nc.sync.dma_start(attn_t[t], e_tile[:])
                    a_tiles[t] = e_tile

                # ---------------- context^T accumulation for the pair ----------------
                a0, a1 = a_tiles[2 * g], a_tiles[2 * g + 1]
                ctx_ps = cpsum.tile([P, 256], f32, tag="ctx_ps")
                for cp in range(N_CH // 2):
                    c0, c1 = 2 * cp, 2 * cp + 1
                    tp = tpsum.tile([P, 512], f32, tag="tp_r")
                    tpf = tp[:].bitcast(f32)
                    nc.tensor.transpose(tpf[:, 0:P], a0[:, c0 * P : (c0 + 1) * P], ident[:])
                    nc.tensor.transpose(tpf[:, P : 2 * P], a1[:, c0 * P : (c0 + 1) * P], ident[:])
                    nc.tensor.transpose(tpf[:, 2 * P : 3 * P], a0[:, c1 * P : (c1 + 1) * P], ident[:])
                    nc.tensor.transpose(tpf[:, 3 * P : 4 * P], a1[:, c1 * P : (c1 + 1) * P], ident[:])
                    stg = stage_p.tile([P, 512], f32r, tag="stage")
                    nc.vector.tensor_copy(stg[:], tpf[:])
                    nc.tensor.matmul(
                        ctx_ps[:],
                        v_nat_r[:, c0, :],
                        stg[:, 0:256],
                        start=(c0 == 0),
                        stop=False,
                    )
                    nc.tensor.matmul(
                        ctx_ps[:],
                        v_nat_r[:, c1, :],
                        stg[:, 256:512],
                        start=False,
                        stop=(c1 == N_CH - 1),
                    )
                # ctx_ps = context^T [d, 256] -> transpose back to [m, d]
                ctxT_sb = small.tile([P, 256], f32, tag="ctxT_sb")
                nc.scalar.copy(ctxT_sb[:], ctx_ps[:])
                tp2 = tpsum.tile([P, 512], f32, tag="tp_r")
                tp2f = tp2[:].bitcast(f32)
                nc.tensor.transpose(tp2f[:, 0:P], ctxT_sb[:, 0:P], ident[:])
                nc.tensor.transpose(tp2f[:, P : 2 * P], ctxT_sb[:, P : 2 * P], ident[:])
                ctx_sb = small.tile([P, 256], f32, tag="ctx_sb")
                nc.vector.tensor_copy(ctx_sb[:], tp2f[:, 0 : 2 * P])
                nc.sync.dma_start(ctx_t[2 * g], ctx_sb[:, 0:P])
                nc.sync.dma_start(ctx_t[2 * g + 1], ctx_sb[:, P : 2 * P])


_CACHE = {}


def _build(repeat=1):
    key = repeat
    if key in _CACHE:
        return _CACHE[key]
    nc = bacc.Bacc(
        "TRN2", target_bir_lowering=False, debug=False, num_devices=N_CORES
    )
    q = nc.dram_tensor("q", [M_SH, D], f32, kind="ExternalInput").ap()
    v = nc.dram_tensor("v", [N, D], f32, kind="ExternalInput").ap()
    attn = nc.dram_tensor("attn", [M_SH, N], f32, kind="ExternalOutput").ap()
    ctx_o = nc.dram_tensor("ctx", [M_SH, D], f32, kind="ExternalOutput").ap()
    with tile.TileContext(nc) as tc:
        _attention(tc, q, v, attn, ctx_o, repeat=repeat)
    nc.compile()
    _CACHE[key] = nc
    return nc


def _run(query, value, repeat=1):
    query = np.ascontiguousarray(np.asarray(query, dtype=np.float32))
    value = np.ascontiguousarray(np.asarray(value, dtype=np.float32))
    nc = _build(repeat)
    in_maps = []
    for c in range(N_CORES):
        in_maps.append(
            {
                "q": query[c * M_SH : (c + 1) * M_SH],
                "v": np.ascontiguousarray(np.roll(value, -c * M_SH, axis=0)),
            }
        )
    res = run_bass_kernel_spmd(nc, in_maps, list(range(N_CORES)))
    attn_full = np.empty((N, N), dtype=np.float32)
    ctx_full = np.empty((N, D), dtype=np.float32)
    for c in range(N_CORES):
        attn_full[c * M_SH : (c + 1) * M_SH] = np.roll(
            res.results[c]["attn"], c * M_SH, axis=1
        )
        ctx_full[c * M_SH : (c + 1) * M_SH] = res.results[c]["ctx"]
    return ctx_full, attn_full


def kernel(query, value):
    return _run(query, value, repeat=1)


# revision 22
# speedup vs baseline: 1617.7696x; 1617.7696x over previous
"""Row-parallel masked-diagonal attention for Trainium2 (8 NeuronCores).

Problem: nn_DotProductAttention — N=8192, D=128 fp32.
    score   = (Q @ V^T) * (1 - eye(N))
    attn    = softmax(score, axis=-1)
    context = attn @ V
    returns (context, attn)

Strategy (per core c, rows [1024c, 1024c+1024)):
  - Host rotates V by -1024c rows for core c, so the diagonal of the score
    block always falls at static column offset 128*t for m-tile t (the host
    un-rotates attn columns afterwards; context is rotation-invariant).
  - S = Q V^T computed in 3 float32r passes (Qr·Vr + dQ·Vr + Qr·dV), which
    recovers ~fp32 accuracy at 1 PE cycle/row instead of fp32's 4.
  - exp on ACT directly from PSUM with accum_out giving row sums for free.
  - normalize in place on DVE (per-partition 1/rowsum).
  - A^T tiles via PE transpose (fp32) -> PSUM -> DVE copy (rounds to f32r)
  - context^T accumulated over 64 n-chunks in PSUM (f32r matmuls, free dim
    256 so they run at 1 cycle/row), then PE-transposed back to [m, d].
"""

import sys

sys.path.insert(0, "/opt/trn_rl_repo")

from contextlib import ExitStack

import numpy as np

import concourse.bass as bass
import concourse.tile as tile
from concourse import bacc, mybir
from concourse.bass_utils import run_bass_kernel_spmd
from concourse.masks import make_identity

N_CORES = 8
N, D = 8192, 128
P = 128
M_SH = N // N_CORES  # 1024 rows per core
N_MT = M_SH // P  # 8 m-tiles per core
N_CH = N // P  # 64 n-chunks
N_JT = N // 1024  # 8 jtiles (1024-wide S psum tiles)

f32 = mybir.dt.float32
f32r = mybir.dt.float32r
EXP = mybir.ActivationFunctionType.Exp
MUL = mybir.AluOpType.mult
SUB = mybir.AluOpType.subtract
AXX = mybir.AxisListType.X


def _attention(tc, qT, vsw, vT, attn, ctx_out, repeat=1):
    nc = tc.nc
    ctx = ExitStack()
    with ctx:
        persist = ctx.enter_context(tc.tile_pool(name="persist", bufs=1))
        big = ctx.enter_context(tc.tile_pool(name="big", bufs=2))
        stage_p = ctx.enter_context(tc.tile_pool(name="stage", bufs=4))
        small = ctx.enter_context(tc.tile_pool(name="small", bufs=2))
        consts = ctx.enter_context(tc.tile_pool(name="consts", bufs=1))
        spsum = ctx.enter_context(tc.tile_pool(name="spsum", bufs=2, space="PSUM"))
        tpsum = ctx.enter_context(tc.tile_pool(name="tpsum", bufs=3, space="PSUM"))
        cpsum = ctx.enter_context(tc.tile_pool(name="cpsum", bufs=1, space="PSUM"))

        # ---------------- constants ----------------
        ident = consts.tile([P, P], f32, tag="ident")
        make_identity(nc, ident[:])
        ident_r = consts.tile([P, P], f32r, tag="ident_r")
        nc.vector.tensor_copy(ident_r[:], ident[:])
        one_m_i = consts.tile([P, P], f32, tag="onemi")
        nc.vector.memset(one_m_i[:], 1.0)
        nc.vector.tensor_tensor(one_m_i[:], one_m_i[:], ident[:], SUB)

        # Setup loads are staged through small per-group temps (NOT the big
        # pool, whose 2 slots must stay free for the first e_tiles).
        vTr = persist.tile([P, N], f32r, tag="vTr")
        dvT = persist.tile([P, N], f32r, tag="dvT")
        qTr = persist.tile([P, M_SH], f32r, tag="qTr")
        dqT = persist.tile([P, M_SH], f32r, tag="dqT")
        v_nat_r = persist.tile([P, N], f32r, tag="v_nat_r")
        VG = 1024  # columns per pipeline group

        def load_split(dram_cols, dst_r, dst_d, ns):
            tmp = small.tile([P, VG], f32, tag="setup_tmp")
            nc.sync.dma_start(tmp[:], dram_cols)
            nc.vector.tensor_copy(dst_r[:, ns], tmp[:])
            if dst_d is not None:
                nc.vector.tensor_tensor(
                    dst_d[:, ns], tmp[:], dst_r[:, ns].bitcast(f32), SUB
                )

        # group-load order matches the first m-tile's jt order (1,2,...,7,0)
        load_split(vT[:, VG : 2 * VG], vTr, dvT, slice(VG, 2 * VG))
        load_split(qT[:], qTr, dqT, slice(0, M_SH))
        for gidx in [2, 3, 4, 5, 6, 7, 0]:
            n0 = gidx * VG
            load_split(vT[:, n0 : n0 + VG], vTr, dvT, slice(n0, n0 + VG))
        # V swizzled (host: row p = concat_c V[128c+p, :]): column block
        # c*128..(c+1)*128 of row p is V-chunk row [n=128c+p, d]
        for n0 in range(0, N, VG):
            load_split(vsw[:, n0 : n0 + VG], v_nat_r, None, slice(n0, n0 + VG))

        attn_t = attn.rearrange("(t p) n -> t p n", p=P)
        ctx_t = ctx_out.rearrange("(t p) d -> t p d", p=P)

        for _rep in range(repeat):
            a_tiles = {}
            rinvs = {}
            for g in range(N_MT // 2):  # m-pair groups
                for t in (2 * g, 2 * g + 1):
                    # ---------------- S = Q V^T (3-pass f32r) ----------------
                    e_tile = big.tile([P, N], f32, tag="bigbuf")
                    partials = small.tile([P, N_JT], f32, tag="partials")
                    qsl = qTr[:, t * P : (t + 1) * P]
                    dsl = dqT[:, t * P : (t + 1) * P]
                    # jt 0 carries the diagonal-mask DVE op; do it last so the
                    # mask never gates the exp/psum-recycle chain.
                    for jt in list(range(1, N_JT)) + [0]:
                        s_ps = spsum.tile([P, 1024], f32, tag="s_ps")
                        sls = [s_ps[:, j * 512 : (j + 1) * 512] for j in range(2)]
                        cols = [
                            slice(jt * 1024 + j * 512, jt * 1024 + (j + 1) * 512)
                            for j in range(2)
                        ]
                        # order keeps the qTr stationary resident for 4 mms
                        for j in range(2):
                            nc.tensor.matmul(
                                sls[j], qsl, vTr[:, cols[j]], start=True, stop=False
                            )
                            nc.tensor.matmul(
                                sls[j], qsl, dvT[:, cols[j]], start=False, stop=False
                            )
                        for j in range(2):
                            nc.tensor.matmul(
                                sls[j], dsl, vTr[:, cols[j]], start=False, stop=True
                            )
                        if jt == 0:
                            # zero the diagonal block (rows t*128+p, col == row)
                            blk = s_ps[:, t * P : (t + 1) * P]
                            nc.vector.tensor_tensor(blk, blk, one_m_i[:], MUL)
                        nc.scalar.activation(
                            e_tile[:, jt * 1024 : (jt + 1) * 1024],
                            s_ps[:],
                            EXP,
                            accum_out=partials[:, jt : jt + 1],
                        )
                    # ---------------- softmax row sums ----------------
                    rowsum = small.tile([P, 1], f32, tag="rowsum")
                    nc.vector.reduce_sum(rowsum[:], partials[:], axis=AXX)
                    rinv = small.tile([P, 1], f32, tag="rinv")
                    nc.vector.reciprocal(rinv[:], rowsum[:])
                    a_tiles[t] = e_tile
                    rinvs[t] = rinv

                # ---------------- normalize + context^T for the pair --------
                # normalize is interleaved jtile-by-jtile with the A^T
                # transposes/copies so the first context matmuls don't queue
                # behind all 8 normalize ops on DVE. Normalize writes f32r
                # bits in place (valid fp32, ~2^-13 rel rounding) so the
                # transposes run at 1.5 cyc/row.
                a0, a1 = a_tiles[2 * g], a_tiles[2 * g + 1]
                ctx_ps = cpsum.tile([P, 256], f32, tag="ctx_ps")
                stages = {}

                def produce(cp):
                    if cp % 4 == 0:
                        jt = cp // 4
                        sl = slice(jt * 1024, (jt + 1) * 1024)
                        for t in (2 * g, 2 * g + 1):
                            e_tile = a_tiles[t]
                            nc.vector.tensor_scalar_mul(
                                e_tile[:, sl], e_tile[:, sl], rinvs[t][:]
                            )
                        for t in (2 * g, 2 * g + 1):
                            nc.sync.dma_start(attn_t[t][:, sl], a_tiles[t][:, sl])
                    c0, c1 = 2 * cp, 2 * cp + 1
                    tp = tpsum.tile([P, 512], f32, tag="tp_r")
                    for k, (a, c) in enumerate(
                        ((a0, c0), (a1, c0), (a0, c1), (a1, c1))
                    ):
                        nc.tensor.transpose(
                            tp[:, k * P : (k + 1) * P],
                            a[:, c * P : (c + 1) * P],
                            ident[:],
                        )
                    stg = stage_p.tile([P, 512], f32r, tag="stage")
                    if cp % 4 < 2:
                        nc.scalar.copy(stg[:], tp[:])
                    else:
                        nc.vector.tensor_copy(stg[:], tp[:])
                    stages[cp] = stg

                def consume(cp):
                    stg = stages.pop(cp)
                    c0, c1 = 2 * cp, 2 * cp + 1
                    nc.tensor.matmul(
                        ctx_ps[:],
                        v_nat_r[:, c0 * P : (c0 + 1) * P],
                        stg[:, 0:256],
                        start=(c0 == 0),
                        stop=False,
                    )
                    nc.tensor.matmul(
                        ctx_ps[:],
                        v_nat_r[:, c1 * P : (c1 + 1) * P],
                        stg[:, 256:512],
                        start=False,
                        stop=(c1 == N_CH - 1),
                    )

                # 2-deep software pipeline: transposes run ahead of the ctx
                # matmuls so the PSUM->SBUF stage-copy latency is hidden.
                NCP = N_CH // 2
                for cp in range(NCP):
                    produce(cp)
                    if cp >= 2:
                        consume(cp - 2)
                consume(NCP - 2)
                consume(NCP - 1)
                # ctx_ps = context^T [d, 256] -> transpose back to [m, d]
                ctxT_sb = small.tile([P, 256], f32, tag="ctxT_sb")
                nc.scalar.copy(ctxT_sb[:], ctx_ps[:])
                tp2 = tpsum.tile([P, 512], f32, tag="tp_r")
                tp2f = tp2[:].bitcast(f32)
                nc.tensor.transpose(tp2f[:, 0:P], ctxT_sb[:, 0:P], ident[:])
                nc.tensor.transpose(tp2f[:, P : 2 * P], ctxT_sb[:, P : 2 * P], ident[:])
                ctx_sb = small.tile([P, 256], f32, tag="ctx_sb")
                nc.vector.tensor_copy(ctx_sb[:], tp2f[:, 0 : 2 * P])
                nc.sync.dma_start(ctx_t[2 * g], ctx_sb[:, 0:P])
                nc.sync.dma_start(ctx_t[2 * g + 1], ctx_sb[:, P : 2 * P])


_CACHE = {}


def _build(repeat=1):
    key = repeat
    if key in _CACHE:
        return _CACHE[key]
    nc = bacc.Bacc(
        "TRN2", target_bir_lowering=False, debug=False, num_devices=N_CORES
    )
    qT = nc.dram_tensor("qT", [D, M_SH], f32, kind="ExternalInput").ap()
    vsw = nc.dram_tensor("vsw", [P, N], f32, kind="ExternalInput").ap()
    vT = nc.dram_tensor("vT", [D, N], f32, kind="ExternalInput").ap()
    attn = nc.dram_tensor("attn", [M_SH, N], f32, kind="ExternalOutput").ap()
    ctx_o = nc.dram_tensor("ctx", [M_SH, D], f32, kind="ExternalOutput").ap()
    with tile.TileContext(nc) as tc:
        _attention(tc, qT, vsw, vT, attn, ctx_o, repeat=repeat)
    nc.compile()
    _CACHE[key] = nc
    return nc


def _run(query, value, repeat=1):
    query = np.ascontiguousarray(np.asarray(query, dtype=np.float32))
    value = np.ascontiguousarray(np.asarray(value, dtype=np.float32))
    nc = _build(repeat)
    in_maps = []
    for c in range(N_CORES):
        v_rot = np.roll(value, -c * M_SH, axis=0)
        vsw = np.ascontiguousarray(
            v_rot.reshape(N_CH, P, D).transpose(1, 0, 2).reshape(P, N)
        )
        in_maps.append(
            {
                "qT": np.ascontiguousarray(query[c * M_SH : (c + 1) * M_SH].T),
                "vsw": vsw,
                "vT": np.ascontiguousarray(v_rot.T),
            }
        )
    res = run_bass_kernel_spmd(nc, in_maps, list(range(N_CORES)))
    attn_full = np.empty((N, N), dtype=np.float32)
    ctx_full = np.empty((N, D), dtype=np.float32)
    for c in range(N_CORES):
        attn_full[c * M_SH : (c + 1) * M_SH] = np.roll(
            res.results[c]["attn"], c * M_SH, axis=1
        )
        ctx_full[c * M_SH : (c + 1) * M_SH] = res.results[c]["ctx"]
    return ctx_full, attn_full


def kernel(query, value):
    return _run(query, value, repeat=1)
